# revision 5
# baseline (speedup 1.0000x reference)
"""GCNConv (dense adjacency, 8192 nodes, 512 feat) on 8 Trainium2 NeuronCores.

Math (matches reference):
    A = adj + I
    deg = A.sum(axis=1); dinv = rsqrt(deg)        (deg >= 1 always)
    h = concat(x[:4096] @ Wr, x[4096:] @ Wd)
    out = leaky_relu(dinv[:,None] * (A @ (dinv[:,None] * h)) + bias, 0.01)

Sharding: rows of A / output row-sharded over 8 cores (1024 rows each).

v2 dataflow (vs the v1 baseline):
  - ONE adjacency shipment per core: adjt8 = A[rows].T as fp8e4 codes
    (0/1/2 exact), [64, 128, 1024], resident in SBUF (64KB/partition).
    Serves BOTH the degree pass (fp8 DoubleRow, viewing adjacent k-tile
    pairs as the [128,2,N] DR moving operand) AND the main matmul, as the
    moving operand of a mixed-dtype bf16(stationary) x fp8e4(moving)
    matmul -- verified bit-exact on HW. Halves HBM traffic vs v1's
    bf16+fp8 double shipment.
  - deg/dinv never leave the core: each core scales its OWN h rows by its
    OWN dinv (g = dinv*h) BEFORE the gather, so v1's deg AllGather, the
    all-nodes dinv pass, the PE transpose, and the 8.4M-element post-
    gather scale all disappear. The epilogue's i-scaling (own rows) is a
    free-axis broadcast built with a K=1 ones-matmul, as in v1.
  - g is AllGathered in 8 per-row-tile chunks, each scaled+bounced as
    soon as possible, so the main matmul starts after the first chunk
    lands and the rest of the collective hides behind PE work.
  - g_t k-axis is slice-major: slot s*8+c' holds global j-tile c'*8+s, so
    each sub-AllGather fills a contiguous [128, 8, 512] block; the main
    matmul pairs g_t slot t with adjacency k-tile (t%8)*8 + t//8.
  - main matmul computed transposed (out.T = g.T @ A.T): stationary is a
    [128,128] feature-chunk of g, moving a [128,512] half of an adjacency
    k-tile; bias becomes per-partition and fuses into the LeakyReLU
    activation; the last slice runs cc-major so each chunk's epilogue
    overlaps the next chunk's matmuls.
"""

import numpy as np
import ml_dtypes

import concourse.bass as bass
import concourse.tile as tile
from concourse.masks import make_identity
from concourse import bacc, mybir
from concourse.bass_utils import run_bass_kernel_spmd

N = 8192
C = 512
NCORES = 8
ROWS = N // NCORES       # 1024 rows per core
P = 128
KT = N // P              # 64 global j-tiles
MT = ROWS // P           # 8 own row tiles (= AllGather slices)
FT = C // P              # 4 feature tiles for x @ W
CC = C // P              # 4 feature chunks (stationary side of main matmul)
NDQ = 8                  # adjacency DMA chunks

F32 = mybir.dt.float32
BF16 = mybir.dt.bfloat16
FP8 = mybir.dt.float8e4

DR = mybir.MatmulPerfMode.DoubleRow


def _emit(nc, tc, dram, io, r, parts="all", nch=MT, nf8=0, sched="late"):
    adjt8_d, xt_d, w_d, biasc_d, out_d = io

    with tc.tile_pool(name=f"const{r}", bufs=1) as const_pool, \
         tc.tile_pool(name=f"misc{r}", bufs=1) as misc_pool, \
         tc.tile_pool(name=f"adj{r}", bufs=1) as adj_pool, \
         tc.tile_pool(name=f"gt{r}", bufs=1) as gt_pool:
        bias_pp = const_pool.tile([P, CC], F32)
        nc.sync.dma_start(bias_pp[:],
                          biasc_d.ap().rearrange("(cc p) -> p cc", p=P))
        dinvr_bc = const_pool.tile([P, ROWS], F32)

        # adjacency, resident: adj_sb[p, kt, i] = A.T[kt*128+p, i]
        adj_sb = adj_pool.tile([P, KT, ROWS], FP8)
        for q in range(NDQ):
            kq = KT // NDQ
            nc.sync.dma_start(
                adj_sb[:, q * kq:(q + 1) * kq, :],
                adjt8_d.ap()[q * kq:(q + 1) * kq].rearrange("k p i -> p k i"))
        # g, slice-major: slot s*8+c' = global j-tile c'*8+s
        g_t = gt_pool.tile([P, KT, C], BF16)

        if parts == "mm":
            # timing isolation: fill g_t with xt bytes (benign), unit scales
            for b in range(8):
                nc.sync.dma_start(
                    g_t[:, 8 * b:8 * (b + 1), :].rearrange(
                        "p k c -> p (k c)").rearrange("p (f i) -> p f i", f=FT),
                    xt_d.ap().rearrange("(f p) i -> p f i", p=P))
            nc.vector.memset(dinvr_bc[:], 1.0)
        else:
            dinv_all = _emit_pre(nc, tc, dram, io, r, g_t, dinvr_bc,
                                 misc_pool, adj_sb, nch, sched)
            if parts == "pre":
                tok = misc_pool.tile([P, C], F32, tag="tok")
                nc.vector.tensor_copy(tok[:], g_t[:, KT - 1, :])
                nc.sync.dma_start(out_d.ap()[0:P, 0:C], tok[:])
                return

        # ---------------- main matmul + epilogue -----------------------
        stiles = ROWS // nch // P
        cht = NCORES * stiles                  # g_t slots per AllGather chunk
        t8lo = KT - nf8                        # first fp8 slot (nf8 k-tiles)
        assert nf8 % 2 == 0 and (parts == "mm" or stiles % 2 == 0 or
                                 nf8 == 0), "fp8 slots must pair up"

        def slot_kt(t):
            # g_t slot t -> global j-tile (= adjacency k-tile index)
            if parts == "mm":
                return t
            s, rem = divmod(t, cht)
            cp, j = divmod(rem, stiles)
            return cp * MT + s * stiles + j

        if sched == "early" and parts != "mm":
            # post-gather k-scaling: g = dinv_j * h, per slot, DVE runs
            # ahead of the PE's consumption
            for t in range(KT):
                nc.vector.tensor_scalar_mul(
                    g_t[:, t, :], g_t[:, t, :],
                    dinv_all[:, slot_kt(t):slot_kt(t) + 1])

        g8_t = None
        if nf8:
            g8_t = gt_pool.tile([P, (KT - t8lo) // 2, 2, C], FP8)
            for u in range((KT - t8lo) // 2):
                nc.vector.tensor_copy(g8_t[:, u, :, :],
                                      g_t[:, t8lo + 2 * u:t8lo + 2 * u + 2, :])

        with tc.tile_pool(name=f"mmps{r}", bufs=1, space="PSUM") as mmps_pool, \
             tc.tile_pool(name=f"ep{r}", bufs=4) as ep_pool:
            mm_ps = [mmps_pool.tile([P, ROWS], F32, tag=f"mm{cc}",
                                    name=f"mm{cc}")
                     for cc in range(CC)]

            def mm(cc, t, start, stop):
                kt = slot_kt(t)
                for half in range(2):
                    nc.tensor.matmul(
                        mm_ps[cc][:, half * C:(half + 1) * C],
                        lhsT=g_t[:, t, cc * P:(cc + 1) * P],
                        rhs=adj_sb[:, kt, half * C:(half + 1) * C],
                        start=start, stop=stop)

            def mm8(cc, t, start, stop):
                # fp8 DoubleRow over slot pair (t, t+1); adjacency k-tiles
                # (kt, kt+1) are consecutive because stiles is even
                kt = slot_kt(t)
                assert slot_kt(t + 1) == kt + 1
                u = (t - t8lo) // 2
                for half in range(2):
                    nc.tensor.matmul(
                        mm_ps[cc][:, half * C:(half + 1) * C],
                        lhsT=g8_t[:, u, :, cc * P:(cc + 1) * P],
                        rhs=adj_sb[:, kt:kt + 2, half * C:(half + 1) * C],
                        perf_mode=DR,
                        start=start, stop=stop)

            tail0 = min(t8lo, KT - MT)
            for t in range(tail0):
                for cc in range(CC):
                    mm(cc, t, start=(t == 0), stop=False)

            # tail: cc-major so epilogues overlap remaining matmuls
            for cc in range(CC):
                t = tail0
                while t < KT:
                    if nf8 and t >= t8lo:
                        mm8(cc, t, start=False, stop=(t + 2 == KT))
                        t += 2
                    else:
                        mm(cc, t, start=False, stop=(t + 1 == KT))
                        t += 1
                for eh in range(2):
                    sl = slice(eh * C, (eh + 1) * C)
                    t1 = ep_pool.tile([P, C], F32, tag="t1")
                    nc.vector.tensor_mul(t1[:], mm_ps[cc][:, sl],
                                         dinvr_bc[:, sl])
                    t2 = ep_pool.tile([P, C], F32, tag="t2")
                    nc.scalar.activation(
                        t2[:], t1[:], mybir.ActivationFunctionType.Lrelu,
                        bias=bias_pp[:, cc:cc + 1], alpha=0.01)
                    nc.sync.dma_start(
                        out_d.ap()[cc * P:(cc + 1) * P, sl], t2[:])


def _emit_pre(nc, tc, dram, io, r, g_t, dinvr_bc, misc_pool, adj_sb, nch,
              sched):
    """x@W, degree pass, dinv, chunked AllGather -> g_t.

    sched="late": scale own h by own dinv, then gather g (fewest moving
    parts; the first gather chunk waits for the full degree pass).
    sched="early": gather UNSCALED h starting right after x@W, slot the
    tiny dinv AllGather after the first h chunk, and return the all-nodes
    dinv [P, KT] so the caller scales g_t per slot post-gather."""
    adjt8_d, xt_d, w_d, biasc_d, out_d = io
    srows = ROWS // nch                # own rows per AllGather chunk
    stiles = srows // P                # own row tiles per chunk

    g_bounce = dram.tile([ROWS, C], BF16, name=f"g_bounce{r}")
    g_alls = [dram.tile([NCORES * srows, C], BF16, addr_space="Shared",
                        name=f"g_all{r}_{s}") for s in range(nch)]
    dinv_b = dram.tile([1, ROWS], F32, name=f"dinv_b{r}")

    ones8_t = misc_pool.tile([P, 2, 16], FP8, tag="ones8")
    nc.gpsimd.memset(ones8_t[:], 1.0)
    ones1_t = misc_pool.tile([1, P], F32, tag="ones1")
    nc.gpsimd.memset(ones1_t[:], 1.0)
    h_sb = misc_pool.tile([P, MT, C], BF16, tag="hsb")

    with tc.tile_pool(name=f"xw{r}", bufs=1) as xw_pool, \
         tc.tile_pool(name=f"hps{r}", bufs=2, space="PSUM") as hps_pool, \
         tc.tile_pool(name=f"degps{r}", bufs=1, space="PSUM") as degps_pool, \
         tc.tile_pool(name=f"bcps{r}", bufs=1, space="PSUM") as bcps_pool:
        # ---------------- x@W ------------------------------------------
        xt_t = xw_pool.tile([P, FT, ROWS], BF16)
        nc.sync.dma_start(
            xt_t[:], xt_d.ap().rearrange("(f p) i -> p f i", p=P))
        w_t = xw_pool.tile([P, FT, C], BF16)
        nc.sync.dma_start(
            w_t[:], w_d.ap().rearrange("(f p) c -> p f c", p=P))

        for mt in range(MT):
            h_ps = hps_pool.tile([P, C], F32)
            for ft in range(FT):
                nc.tensor.matmul(
                    h_ps[:],
                    lhsT=xt_t[:, ft, mt * P:(mt + 1) * P],
                    rhs=w_t[:, ft, :],
                    start=(ft == 0), stop=(ft == FT - 1))
            nc.scalar.copy(h_sb[:, mt, :], h_ps[:])

        def em_ag(s):
            nc.gpsimd.collective_compute(
                "AllGather", mybir.AluOpType.bypass,
                replica_groups=[list(range(NCORES))],
                ins=[g_bounce[s * srows:(s + 1) * srows, :].opt()],
                outs=[g_alls[s].opt()])
            nc.sync.dma_start(
                g_t[:, s * NCORES * stiles:(s + 1) * NCORES * stiles, :],
                g_alls[s].rearrange("(k p) c -> p k c", p=P))

        if sched == "early":
            # bounce chunk 0 of unscaled h; its gather goes out before the
            # degree pass finishes. Later chunks are dependency-chained
            # behind the dinv AllGather (see below) so the tiny dinv
            # collective gets the 2nd FIFO slot, not the last.
            nc.sync.dma_start(
                g_bounce[0:srows, :].rearrange("(m p) c -> p m c", p=P),
                h_sb[:, 0:stiles, :])
            em_ag(0)

        # ---------------- deg: fp8 DoubleRow over resident adjacency ---
        deg_ps = [degps_pool.tile([1, C], F32, tag=f"degp{i}", name=f"degp{i}")
                  for i in range(2)]
        for q in range(KT // 2):
            for half in range(2):
                nc.tensor.matmul(
                    deg_ps[half][:],
                    lhsT=ones8_t[:, :, 0:1],
                    rhs=adj_sb[:, 2 * q:2 * q + 2, half * C:(half + 1) * C],
                    perf_mode=DR,
                    start=(q == 0), stop=(q == KT // 2 - 1))

        deg_sb = misc_pool.tile([1, ROWS], F32, tag="degsb")
        for half in range(2):
            nc.vector.tensor_copy(
                deg_sb[:, half * C:(half + 1) * C], deg_ps[half][:])
        rrow = misc_pool.tile([1, ROWS], F32, tag="rrow")
        nc.vector.reciprocal(rrow[:], deg_sb[:])
        drow = misc_pool.tile([1, ROWS], F32, tag="drow")
        nc.scalar.sqrt(drow[:], rrow[:])

        # free-axis broadcast of own dinv (epilogue i-scaling)
        bc_ps = bcps_pool.tile([P, ROWS], F32)
        for half in range(2):
            nc.tensor.matmul(
                bc_ps[:, half * C:(half + 1) * C],
                lhsT=ones1_t[:],
                rhs=drow[:, half * C:(half + 1) * C],
                start=True, stop=True)
        nc.vector.tensor_copy(dinvr_bc[:], bc_ps[:])

        if sched == "early":
            # tiny dinv AllGather slots in after the first h chunk, then
            # the remaining h chunks; all-nodes dinv -> [P, KT] via a
            # natural [KT, P] load + PE transpose.
            # dinv_b row 1 is junk copied from the first h chunk's OUTPUT:
            # a real data dependency that forces the scheduler to place
            # this collective after that chunk on the FIFO queue (priority
            # hints alone are ignored).
            dinv_b2 = dram.tile([2, ROWS], F32, name=f"dinv_b2_{r}")
            nc.sync.dma_start(dinv_b2[0:1, :], drow[:])
            nc.gpsimd.dma_start(dinv_b2[1:2, 0:C // 2],
                                g_alls[0][0:1, 0:C // 2])
            dinv_all_b = dram.tile([NCORES * 2, ROWS], F32,
                                   addr_space="Shared", name=f"dinv_all{r}")
            nc.gpsimd.collective_compute(
                "AllGather", mybir.AluOpType.bypass,
                replica_groups=[list(range(NCORES))],
                ins=[dinv_b2.opt()], outs=[dinv_all_b.opt()])
            for s in range(1, nch):
                # junk write from the dinv gather's output into this
                # chunk's bounce region (immediately overwritten by the
                # real bounce): forces chunk s behind the dinv collective
                nc.gpsimd.dma_start(
                    g_bounce[s * srows:s * srows + 1, 0:2],
                    dinv_all_b[0:1, 0:2])
                nc.sync.dma_start(
                    g_bounce[s * srows:(s + 1) * srows, :].rearrange(
                        "(m p) c -> p m c", p=P),
                    h_sb[:, s * stiles:(s + 1) * stiles, :])
                em_ag(s)

            dinvkt_sb = misc_pool.tile([KT, P], F32, tag="dinvkt")
            for cp in range(NCORES):
                nc.sync.dma_start(
                    dinvkt_sb[cp * MT:(cp + 1) * MT, :],
                    dinv_all_b[2 * cp:2 * cp + 1, :].rearrange(
                        "one (m p) -> (one m) p", p=P))
            ident_t = misc_pool.tile([KT, KT], F32, tag="ident")
            make_identity(nc, ident_t[:])
            tp_ps = bcps_pool.tile([P, KT], F32, tag="tp")
            nc.tensor.transpose(tp_ps[:], dinvkt_sb[:], ident_t[:])
            dinv_all = misc_pool.tile([P, KT], F32, tag="dinvall")
            nc.vector.tensor_copy(dinv_all[:], tp_ps[:])
            return dinv_all

        # sched == "late":
        # per-partition dinv for the k-scaling of own h rows:
        # dinv_pm[p, mt] = drow[mt*128 + p] via a DRAM round-trip
        nc.sync.dma_start(dinv_b[:], drow[:])
        dinv_pm = misc_pool.tile([P, MT], F32, tag="dinvpm")
        nc.sync.dma_start(
            dinv_pm[:],
            dinv_b.rearrange("one (m p) -> p (one m)", p=P))

        # ---------------- scale + bounce + AllGather, per chunk --------
        for s in range(nch):
            for j in range(stiles):
                mt = s * stiles + j
                nc.vector.tensor_scalar_mul(
                    h_sb[:, mt, :], h_sb[:, mt, :], dinv_pm[:, mt:mt + 1])
            nc.sync.dma_start(
                g_bounce[s * srows:(s + 1) * srows, :].rearrange(
                    "(m p) c -> p m c", p=P),
                h_sb[:, s * stiles:(s + 1) * stiles, :])
            em_ag(s)
        return None


def build_kernel(reps: int = 1, parts: str = "all", nch: int = 4,
                 nf8: int = 24, sched: str = "early"):
    """Build and compile the SPMD Bass program (identical on all 8 cores)."""
    nc = bacc.Bacc("TRN2", target_bir_lowering=False, debug=False,
                   num_devices=NCORES)

    adjt8_d = nc.dram_tensor("adjt8", [KT, P, ROWS], FP8, kind="ExternalInput")
    xt_d = nc.dram_tensor("xt", [C, ROWS], BF16, kind="ExternalInput")
    w_d = nc.dram_tensor("w", [C, C], BF16, kind="ExternalInput")
    biasc_d = nc.dram_tensor("biasc", [C], F32, kind="ExternalInput")
    out_d = nc.dram_tensor("out", [C, ROWS], F32, kind="ExternalOutput")
    io = (adjt8_d, xt_d, w_d, biasc_d, out_d)

    with tile.TileContext(nc) as tc:
        with tc.tile_pool(name="dram", bufs=1, space="DRAM") as dram:
            if reps == 0:
                with tc.tile_pool(name="nullp", bufs=1) as np_pool:
                    z = np_pool.tile([P, CC], F32)
                    nc.sync.dma_start(
                        z[:], biasc_d.ap().rearrange("(cc p) -> p cc", p=P))
            for r in range(reps):
                _emit(nc, tc, dram, io, r, parts=parts, nch=nch, nf8=nf8,
                      sched=sched)

    nc.compile()
    return nc


def prepare_inputs(x, adj, weightr, weightd, bias):
    """Host-side sharding/layout. Returns in_maps for the 8 cores."""
    x = np.asarray(x, dtype=np.float32)
    adj = np.asarray(adj, dtype=np.float32)
    weightr = np.asarray(weightr, dtype=np.float32)
    weightd = np.asarray(weightd, dtype=np.float32)
    bias = np.ascontiguousarray(np.asarray(bias, dtype=np.float32))

    wr16 = weightr.astype(ml_dtypes.bfloat16)
    wd16 = weightd.astype(ml_dtypes.bfloat16)
    idx = np.arange(ROWS)
    # A values are only 0/1/2: build uint8 once, then LUT-cast (fast + exact)
    lut8 = np.array([0x00, 0x38, 0x40], dtype=np.uint8)  # e4m3 bits for 0/1/2

    in_maps = []
    for c in range(NCORES):
        rows = slice(c * ROWS, (c + 1) * ROWS)
        ai = adj[rows, :].T.astype(np.uint8)             # [N, ROWS] 0/1
        ai[c * ROWS + idx, idx] += 1                     # fold in self-loop
        adjt8 = lut8[ai].view(ml_dtypes.float8_e4m3).reshape(KT, P, ROWS)
        xt = np.ascontiguousarray(x[rows, :].T).astype(ml_dtypes.bfloat16)
        w = wr16 if c < NCORES // 2 else wd16
        in_maps.append({"adjt8": adjt8, "xt": xt, "w": w, "biasc": bias})
    return in_maps


_NC_CACHE = {}


def kernel(x, adj, weightr, weightd, bias):
    if "nc" not in _NC_CACHE:
        _NC_CACHE["nc"] = build_kernel(reps=1)
    nc = _NC_CACHE["nc"]
    in_maps = prepare_inputs(x, adj, weightr, weightd, bias)
    res = run_bass_kernel_spmd(nc, in_maps, list(range(NCORES)))
    out = np.concatenate(
        [np.ascontiguousarray(res.results[c]["out"].T) for c in range(NCORES)],
        axis=0)
    return out


# revision 6
# speedup vs baseline: 1.4655x; 1.4655x over previous
"""GCNConv (dense adjacency, 8192 nodes, 512 feat) on 8 Trainium2 NeuronCores.

Math (matches reference):
    A = adj + I
    deg = A.sum(axis=1); dinv = rsqrt(deg)        (deg >= 1 always)
    h = concat(x[:4096] @ Wr, x[4096:] @ Wd)
    out = leaky_relu(dinv[:,None] * (A @ (dinv[:,None] * h)) + bias, 0.01)

Sharding: rows of A / output row-sharded over 8 cores (1024 rows each).

v2 dataflow (vs the v1 baseline):
  - ONE adjacency shipment per core: adjt8 = A[rows].T as fp8e4 codes
    (0/1/2 exact), [64, 128, 1024], resident in SBUF (64KB/partition).
    Serves BOTH the degree pass (fp8 DoubleRow, viewing adjacent k-tile
    pairs as the [128,2,N] DR moving operand) AND the main matmul, as the
    moving operand of a mixed-dtype bf16(stationary) x fp8e4(moving)
    matmul -- verified bit-exact on HW. Halves HBM traffic vs v1's
    bf16+fp8 double shipment.
  - deg/dinv never leave the core: each core scales its OWN h rows by its
    OWN dinv (g = dinv*h) BEFORE the gather, so v1's deg AllGather, the
    all-nodes dinv pass, the PE transpose, and the 8.4M-element post-
    gather scale all disappear. The epilogue's i-scaling (own rows) is a
    free-axis broadcast built with a K=1 ones-matmul, as in v1.
  - g is AllGathered in 8 per-row-tile chunks, each scaled+bounced as
    soon as possible, so the main matmul starts after the first chunk
    lands and the rest of the collective hides behind PE work.
  - g_t k-axis is slice-major: slot s*8+c' holds global j-tile c'*8+s, so
    each sub-AllGather fills a contiguous [128, 8, 512] block; the main
    matmul pairs g_t slot t with adjacency k-tile (t%8)*8 + t//8.
  - main matmul computed transposed (out.T = g.T @ A.T): stationary is a
    [128,128] feature-chunk of g, moving a [128,512] half of an adjacency
    k-tile; bias becomes per-partition and fuses into the LeakyReLU
    activation; the last slice runs cc-major so each chunk's epilogue
    overlaps the next chunk's matmuls.
"""

import numpy as np
import ml_dtypes

import concourse.bass as bass
import concourse.tile as tile
from concourse.masks import make_identity
from concourse import bacc, mybir
from concourse.bass_utils import run_bass_kernel_spmd

N = 8192
C = 512
NCORES = 8
ROWS = N // NCORES       # 1024 rows per core
P = 128
KT = N // P              # 64 global j-tiles
MT = ROWS // P           # 8 own row tiles (= AllGather slices)
FT = C // P              # 4 feature tiles for x @ W
CC = C // P              # 4 feature chunks (stationary side of main matmul)
NDQ = 16                 # adjacency DMA chunks

F32 = mybir.dt.float32
BF16 = mybir.dt.bfloat16
FP8 = mybir.dt.float8e4

DR = mybir.MatmulPerfMode.DoubleRow


def _emit(nc, tc, dram, io, r, parts="all", nch=MT, nf8=0, sched="late"):
    adjt8_d, xt_d, w_d, biasc_d, out_d = io

    with tc.tile_pool(name=f"const{r}", bufs=1) as const_pool, \
         tc.tile_pool(name=f"misc{r}", bufs=1) as misc_pool, \
         tc.tile_pool(name=f"adj{r}", bufs=1) as adj_pool, \
         tc.tile_pool(name=f"gt{r}", bufs=1) as gt_pool:
        bias_pp = const_pool.tile([P, CC], F32)
        nc.sync.dma_start(bias_pp[:],
                          biasc_d.ap().rearrange("(cc p) -> p cc", p=P))
        dinvr_bc = const_pool.tile([P, ROWS], F32)

        # adjacency, resident: adj_sb[p, kt, i] = A.T[kt*128+p, i]
        adj_sb = adj_pool.tile([P, KT, ROWS], FP8)
        for q in range(NDQ):
            kq = KT // NDQ
            nc.sync.dma_start(
                adj_sb[:, q * kq:(q + 1) * kq, :],
                adjt8_d.ap()[q * kq:(q + 1) * kq].rearrange("k p i -> p k i"))
        # g, slice-major: slot s*8+c' = global j-tile c'*8+s
        g_t = gt_pool.tile([P, KT, C], BF16)

        if parts == "mm":
            # timing isolation: fill g_t with xt bytes (benign), unit scales
            for b in range(8):
                nc.sync.dma_start(
                    g_t[:, 8 * b:8 * (b + 1), :].rearrange(
                        "p k c -> p (k c)").rearrange("p (f i) -> p f i", f=FT),
                    xt_d.ap().rearrange("(f p) i -> p f i", p=P))
            nc.vector.memset(dinvr_bc[:], 1.0)
        else:
            dinv_all = _emit_pre(nc, tc, dram, io, r, g_t, dinvr_bc,
                                 misc_pool, adj_sb, nch, sched)
            if parts == "pre":
                tok = misc_pool.tile([P, C], F32, tag="tok")
                nc.vector.tensor_copy(tok[:], g_t[:, KT - 1, :])
                nc.sync.dma_start(out_d.ap()[0:P, 0:C], tok[:])
                return

        # ---------------- main matmul + epilogue -----------------------
        stiles = ROWS // nch // P
        cht = NCORES * stiles                  # g_t slots per AllGather chunk
        t8lo = KT - nf8                        # first fp8 slot (nf8 k-tiles)
        assert nf8 % 2 == 0 and (parts == "mm" or stiles % 2 == 0 or
                                 nf8 == 0), "fp8 slots must pair up"

        def slot_kt(t):
            # g_t slot t -> global j-tile (= adjacency k-tile index)
            if parts == "mm":
                return t
            s, rem = divmod(t, cht)
            cp, j = divmod(rem, stiles)
            return cp * MT + s * stiles + j

        if sched == "early" and parts != "mm":
            # post-gather k-scaling: g = dinv_j * h, per slot, DVE runs
            # ahead of the PE's consumption
            for t in range(KT):
                nc.vector.tensor_scalar_mul(
                    g_t[:, t, :], g_t[:, t, :],
                    dinv_all[:, slot_kt(t):slot_kt(t) + 1])

        g8_t = None
        if nf8:
            g8_t = gt_pool.tile([P, (KT - t8lo) // 2, 2, C], FP8)
            for u in range((KT - t8lo) // 2):
                nc.vector.tensor_copy(g8_t[:, u, :, :],
                                      g_t[:, t8lo + 2 * u:t8lo + 2 * u + 2, :])

        with tc.tile_pool(name=f"mmps{r}", bufs=1, space="PSUM") as mmps_pool, \
             tc.tile_pool(name=f"ep{r}", bufs=4) as ep_pool:
            mm_ps = [mmps_pool.tile([P, ROWS], F32, tag=f"mm{cc}",
                                    name=f"mm{cc}")
                     for cc in range(CC)]

            def mm(cc, t, start, stop):
                kt = slot_kt(t)
                for half in range(2):
                    nc.tensor.matmul(
                        mm_ps[cc][:, half * C:(half + 1) * C],
                        lhsT=g_t[:, t, cc * P:(cc + 1) * P],
                        rhs=adj_sb[:, kt, half * C:(half + 1) * C],
                        start=start, stop=stop)

            def mm8(cc, t, start, stop):
                # fp8 DoubleRow over slot pair (t, t+1); adjacency k-tiles
                # (kt, kt+1) are consecutive because stiles is even
                kt = slot_kt(t)
                assert slot_kt(t + 1) == kt + 1
                u = (t - t8lo) // 2
                for half in range(2):
                    nc.tensor.matmul(
                        mm_ps[cc][:, half * C:(half + 1) * C],
                        lhsT=g8_t[:, u, :, cc * P:(cc + 1) * P],
                        rhs=adj_sb[:, kt:kt + 2, half * C:(half + 1) * C],
                        perf_mode=DR,
                        start=start, stop=stop)

            tail0 = min(t8lo, KT - MT)
            for t in range(tail0):
                for cc in range(CC):
                    mm(cc, t, start=(t == 0), stop=False)

            # tail: cc-major so epilogues overlap remaining matmuls
            for cc in range(CC):
                t = tail0
                while t < KT:
                    if nf8 and t >= t8lo:
                        mm8(cc, t, start=False, stop=(t + 2 == KT))
                        t += 2
                    else:
                        mm(cc, t, start=False, stop=(t + 1 == KT))
                        t += 1
                for eh in range(2):
                    sl = slice(eh * C, (eh + 1) * C)
                    t1 = ep_pool.tile([P, C], F32, tag="t1")
                    nc.vector.tensor_mul(t1[:], mm_ps[cc][:, sl],
                                         dinvr_bc[:, sl])
                    t2 = ep_pool.tile([P, C], F32, tag="t2")
                    nc.scalar.activation(
                        t2[:], t1[:], mybir.ActivationFunctionType.Lrelu,
                        bias=bias_pp[:, cc:cc + 1], alpha=0.01)
                    nc.sync.dma_start(
                        out_d.ap()[cc * P:(cc + 1) * P, sl], t2[:])


def _emit_pre(nc, tc, dram, io, r, g_t, dinvr_bc, misc_pool, adj_sb, nch,
              sched):
    """x@W, degree pass, dinv, chunked AllGather -> g_t.

    sched="late": scale own h by own dinv, then gather g (fewest moving
    parts; the first gather chunk waits for the full degree pass).
    sched="early": gather UNSCALED h starting right after x@W, slot the
    tiny dinv AllGather after the first h chunk, and return the all-nodes
    dinv [P, KT] so the caller scales g_t per slot post-gather."""
    adjt8_d, xt_d, w_d, biasc_d, out_d = io
    srows = ROWS // nch                # own rows per AllGather chunk
    stiles = srows // P                # own row tiles per chunk

    g_bounce = dram.tile([ROWS, C], BF16, name=f"g_bounce{r}")
    g_alls = [dram.tile([NCORES * srows, C], BF16, addr_space="Shared",
                        name=f"g_all{r}_{s}") for s in range(nch)]
    dinv_b = dram.tile([1, ROWS], F32, name=f"dinv_b{r}")

    ones8_t = misc_pool.tile([P, 2, 16], FP8, tag="ones8")
    nc.gpsimd.memset(ones8_t[:], 1.0)
    ones1_t = misc_pool.tile([1, P], F32, tag="ones1")
    nc.gpsimd.memset(ones1_t[:], 1.0)
    h_sb = misc_pool.tile([P, MT, C], BF16, tag="hsb")

    with tc.tile_pool(name=f"xw{r}", bufs=1) as xw_pool, \
         tc.tile_pool(name=f"hps{r}", bufs=2, space="PSUM") as hps_pool, \
         tc.tile_pool(name=f"degps{r}", bufs=1, space="PSUM") as degps_pool, \
         tc.tile_pool(name=f"bcps{r}", bufs=1, space="PSUM") as bcps_pool:
        # HAM warm-up: a few dependency-free matmuls fill the PE during
        # the initial DMA wait, so x@W starts at 2.4GHz instead of 1.2
        warm_ps = bcps_pool.tile([P, P], F32, tag="warm")
        for i in range(12):
            nc.tensor.matmul(warm_ps[:], lhsT=ones1_t[:], rhs=ones1_t[:],
                             start=True, stop=True)

        # ---------------- x@W ------------------------------------------
        xt_t = xw_pool.tile([P, FT, ROWS], BF16)
        nc.sync.dma_start(
            xt_t[:], xt_d.ap().rearrange("(f p) i -> p f i", p=P))
        w_t = xw_pool.tile([P, FT, C], BF16)
        nc.sync.dma_start(
            w_t[:], w_d.ap().rearrange("(f p) c -> p f c", p=P))

        for mt in range(MT):
            h_ps = hps_pool.tile([P, C], F32)
            for ft in range(FT):
                nc.tensor.matmul(
                    h_ps[:],
                    lhsT=xt_t[:, ft, mt * P:(mt + 1) * P],
                    rhs=w_t[:, ft, :],
                    start=(ft == 0), stop=(ft == FT - 1))
            nc.scalar.copy(h_sb[:, mt, :], h_ps[:])

        def em_ag(s):
            nc.gpsimd.collective_compute(
                "AllGather", mybir.AluOpType.bypass,
                replica_groups=[list(range(NCORES))],
                ins=[g_bounce[s * srows:(s + 1) * srows, :].opt()],
                outs=[g_alls[s].opt()])
            nc.sync.dma_start(
                g_t[:, s * NCORES * stiles:(s + 1) * NCORES * stiles, :],
                g_alls[s].rearrange("(k p) c -> p k c", p=P))

        if sched == "early":
            # bounce chunk 0 of unscaled h; its gather goes out before the
            # degree pass finishes. Later chunks are dependency-chained
            # behind the dinv AllGather (see below) so the tiny dinv
            # collective gets the 2nd FIFO slot, not the last.
            nc.sync.dma_start(
                g_bounce[0:srows, :].rearrange("(m p) c -> p m c", p=P),
                h_sb[:, 0:stiles, :])
            em_ag(0)

        # ---------------- deg: fp8 DoubleRow over resident adjacency ---
        deg_ps = [degps_pool.tile([1, C], F32, tag=f"degp{i}", name=f"degp{i}")
                  for i in range(2)]
        for q in range(KT // 2):
            for half in range(2):
                nc.tensor.matmul(
                    deg_ps[half][:],
                    lhsT=ones8_t[:, :, 0:1],
                    rhs=adj_sb[:, 2 * q:2 * q + 2, half * C:(half + 1) * C],
                    perf_mode=DR,
                    start=(q == 0), stop=(q == KT // 2 - 1))

        deg_sb = misc_pool.tile([1, ROWS], F32, tag="degsb")
        for half in range(2):
            nc.vector.tensor_copy(
                deg_sb[:, half * C:(half + 1) * C], deg_ps[half][:])
        rrow = misc_pool.tile([1, ROWS], F32, tag="rrow")
        nc.vector.reciprocal(rrow[:], deg_sb[:])
        drow = misc_pool.tile([1, ROWS], F32, tag="drow")
        nc.scalar.sqrt(drow[:], rrow[:])

        # free-axis broadcast of own dinv (epilogue i-scaling)
        bc_ps = bcps_pool.tile([P, ROWS], F32)
        for half in range(2):
            nc.tensor.matmul(
                bc_ps[:, half * C:(half + 1) * C],
                lhsT=ones1_t[:],
                rhs=drow[:, half * C:(half + 1) * C],
                start=True, stop=True)
        nc.vector.tensor_copy(dinvr_bc[:], bc_ps[:])

        if sched == "early":
            # tiny dinv AllGather slots in after the first h chunk, then
            # the remaining h chunks; all-nodes dinv -> [P, KT] via a
            # natural [KT, P] load + PE transpose.
            # dinv_b row 1 is junk copied from the first h chunk's OUTPUT:
            # a real data dependency that forces the scheduler to place
            # this collective after that chunk on the FIFO queue (priority
            # hints alone are ignored).
            dinv_b2 = dram.tile([2, ROWS], F32, name=f"dinv_b2_{r}")
            nc.sync.dma_start(dinv_b2[0:1, :], drow[:])
            nc.gpsimd.dma_start(dinv_b2[1:2, 0:C // 2],
                                g_alls[0][0:1, 0:C // 2])
            dinv_all_b = dram.tile([NCORES * 2, ROWS], F32,
                                   addr_space="Shared", name=f"dinv_all{r}")
            nc.gpsimd.collective_compute(
                "AllGather", mybir.AluOpType.bypass,
                replica_groups=[list(range(NCORES))],
                ins=[dinv_b2.opt()], outs=[dinv_all_b.opt()])
            for s in range(1, nch):
                # junk write from the dinv gather's output into this
                # chunk's bounce region (immediately overwritten by the
                # real bounce): forces chunk s behind the dinv collective
                nc.gpsimd.dma_start(
                    g_bounce[s * srows:s * srows + 1, 0:2],
                    dinv_all_b[0:1, 0:2])
                nc.sync.dma_start(
                    g_bounce[s * srows:(s + 1) * srows, :].rearrange(
                        "(m p) c -> p m c", p=P),
                    h_sb[:, s * stiles:(s + 1) * stiles, :])
                em_ag(s)

            dinvkt_sb = misc_pool.tile([KT, P], F32, tag="dinvkt")
            for cp in range(NCORES):
                nc.sync.dma_start(
                    dinvkt_sb[cp * MT:(cp + 1) * MT, :],
                    dinv_all_b[2 * cp:2 * cp + 1, :].rearrange(
                        "one (m p) -> (one m) p", p=P))
            ident_t = misc_pool.tile([KT, KT], F32, tag="ident")
            make_identity(nc, ident_t[:])
            tp_ps = bcps_pool.tile([P, KT], F32, tag="tp")
            nc.tensor.transpose(tp_ps[:], dinvkt_sb[:], ident_t[:])
            dinv_all = misc_pool.tile([P, KT], F32, tag="dinvall")
            nc.vector.tensor_copy(dinv_all[:], tp_ps[:])
            return dinv_all

        # sched == "late":
        # per-partition dinv for the k-scaling of own h rows:
        # dinv_pm[p, mt] = drow[mt*128 + p] via a DRAM round-trip
        nc.sync.dma_start(dinv_b[:], drow[:])
        dinv_pm = misc_pool.tile([P, MT], F32, tag="dinvpm")
        nc.sync.dma_start(
            dinv_pm[:],
            dinv_b.rearrange("one (m p) -> p (one m)", p=P))

        # ---------------- scale + bounce + AllGather, per chunk --------
        for s in range(nch):
            for j in range(stiles):
                mt = s * stiles + j
                nc.vector.tensor_scalar_mul(
                    h_sb[:, mt, :], h_sb[:, mt, :], dinv_pm[:, mt:mt + 1])
            nc.sync.dma_start(
                g_bounce[s * srows:(s + 1) * srows, :].rearrange(
                    "(m p) c -> p m c", p=P),
                h_sb[:, s * stiles:(s + 1) * stiles, :])
            em_ag(s)
        return None


def build_kernel(reps: int = 1, parts: str = "all", nch: int = 4,
                 nf8: int = 24, sched: str = "early"):
    """Build and compile the SPMD Bass program (identical on all 8 cores)."""
    nc = bacc.Bacc("TRN2", target_bir_lowering=False, debug=False,
                   num_devices=NCORES)

    adjt8_d = nc.dram_tensor("adjt8", [KT, P, ROWS], FP8, kind="ExternalInput")
    xt_d = nc.dram_tensor("xt", [C, ROWS], BF16, kind="ExternalInput")
    w_d = nc.dram_tensor("w", [C, C], BF16, kind="ExternalInput")
    biasc_d = nc.dram_tensor("biasc", [C], F32, kind="ExternalInput")
    out_d = nc.dram_tensor("out", [C, ROWS], F32, kind="ExternalOutput")
    io = (adjt8_d, xt_d, w_d, biasc_d, out_d)

    with tile.TileContext(nc) as tc:
        with tc.tile_pool(name="dram", bufs=1, space="DRAM") as dram:
            if reps == 0:
                with tc.tile_pool(name="nullp", bufs=1) as np_pool:
                    z = np_pool.tile([P, CC], F32)
                    nc.sync.dma_start(
                        z[:], biasc_d.ap().rearrange("(cc p) -> p cc", p=P))
            for r in range(reps):
                _emit(nc, tc, dram, io, r, parts=parts, nch=nch, nf8=nf8,
                      sched=sched)

    nc.compile()
    return nc


def prepare_inputs(x, adj, weightr, weightd, bias):
    """Host-side sharding/layout. Returns in_maps for the 8 cores."""
    x = np.asarray(x, dtype=np.float32)
    adj = np.asarray(adj, dtype=np.float32)
    weightr = np.asarray(weightr, dtype=np.float32)
    weightd = np.asarray(weightd, dtype=np.float32)
    bias = np.ascontiguousarray(np.asarray(bias, dtype=np.float32))

    wr16 = weightr.astype(ml_dtypes.bfloat16)
    wd16 = weightd.astype(ml_dtypes.bfloat16)
    idx = np.arange(ROWS)
    # A values are only 0/1/2: build uint8 once, then LUT-cast (fast + exact)
    lut8 = np.array([0x00, 0x38, 0x40], dtype=np.uint8)  # e4m3 bits for 0/1/2

    in_maps = []
    for c in range(NCORES):
        rows = slice(c * ROWS, (c + 1) * ROWS)
        ai = adj[rows, :].T.astype(np.uint8)             # [N, ROWS] 0/1
        ai[c * ROWS + idx, idx] += 1                     # fold in self-loop
        adjt8 = lut8[ai].view(ml_dtypes.float8_e4m3).reshape(KT, P, ROWS)
        xt = np.ascontiguousarray(x[rows, :].T).astype(ml_dtypes.bfloat16)
        w = wr16 if c < NCORES // 2 else wd16
        in_maps.append({"adjt8": adjt8, "xt": xt, "w": w, "biasc": bias})
    return in_maps


_NC_CACHE = {}


def kernel(x, adj, weightr, weightd, bias):
    if "nc" not in _NC_CACHE:
        _NC_CACHE["nc"] = build_kernel(reps=1)
    nc = _NC_CACHE["nc"]
    in_maps = prepare_inputs(x, adj, weightr, weightd, bias)
    res = run_bass_kernel_spmd(nc, in_maps, list(range(NCORES)))
    out = np.concatenate(
        [np.ascontiguousarray(res.results[c]["out"].T) for c in range(NCORES)],
        axis=0)
    return out


# revision 7
# speedup vs baseline: 2.2381x; 1.5272x over previous
"""GCNConv (dense adjacency, 8192 nodes, 512 feat) on 8 Trainium2 NeuronCores.

Math (matches reference):
    A = adj + I
    deg = A.sum(axis=1); dinv = rsqrt(deg)        (deg >= 1 always)
    h = concat(x[:4096] @ Wr, x[4096:] @ Wd)
    out = leaky_relu(dinv[:,None] * (A @ (dinv[:,None] * h)) + bias, 0.01)

Sharding: rows of A / output row-sharded over 8 cores (1024 rows each).

v2 dataflow (vs the v1 baseline):
  - ONE adjacency shipment per core: adjt8 = A[rows].T as fp8e4 codes
    (0/1/2 exact), [64, 128, 1024], resident in SBUF (64KB/partition).
    Serves BOTH the degree pass (fp8 DoubleRow, viewing adjacent k-tile
    pairs as the [128,2,N] DR moving operand) AND the main matmul, as the
    moving operand of a mixed-dtype bf16(stationary) x fp8e4(moving)
    matmul -- verified bit-exact on HW. Halves HBM traffic vs v1's
    bf16+fp8 double shipment.
  - deg/dinv never leave the core: each core scales its OWN h rows by its
    OWN dinv (g = dinv*h) BEFORE the gather, so v1's deg AllGather, the
    all-nodes dinv pass, the PE transpose, and the 8.4M-element post-
    gather scale all disappear. The epilogue's i-scaling (own rows) is a
    free-axis broadcast built with a K=1 ones-matmul, as in v1.
  - g is AllGathered in 8 per-row-tile chunks, each scaled+bounced as
    soon as possible, so the main matmul starts after the first chunk
    lands and the rest of the collective hides behind PE work.
  - g_t k-axis is slice-major: slot s*8+c' holds global j-tile c'*8+s, so
    each sub-AllGather fills a contiguous [128, 8, 512] block; the main
    matmul pairs g_t slot t with adjacency k-tile (t%8)*8 + t//8.
  - main matmul computed transposed (out.T = g.T @ A.T): stationary is a
    [128,128] feature-chunk of g, moving a [128,512] half of an adjacency
    k-tile; bias becomes per-partition and fuses into the LeakyReLU
    activation; the last slice runs cc-major so each chunk's epilogue
    overlaps the next chunk's matmuls.
"""

import numpy as np
import ml_dtypes

import concourse.bass as bass
import concourse.tile as tile
from concourse.masks import make_identity
from concourse import bacc, mybir
from concourse.bass_utils import run_bass_kernel_spmd

N = 8192
C = 512
NCORES = 8
ROWS = N // NCORES       # 1024 rows per core
P = 128
KT = N // P              # 64 global j-tiles
MT = ROWS // P           # 8 own row tiles (= AllGather slices)
FT = C // P              # 4 feature tiles for x @ W
CC = C // P              # 4 feature chunks (stationary side of main matmul)
NDQ = 16                 # adjacency DMA chunks

F32 = mybir.dt.float32
BF16 = mybir.dt.bfloat16
FP8 = mybir.dt.float8e4

DR = mybir.MatmulPerfMode.DoubleRow


def _emit(nc, tc, dram, io, r, parts="all", nch=MT, nf8=0, sched="late"):
    adjt8_d, xt_d, w_d, biasc_d, out_d = io

    with tc.tile_pool(name=f"const{r}", bufs=1) as const_pool, \
         tc.tile_pool(name=f"misc{r}", bufs=1) as misc_pool, \
         tc.tile_pool(name=f"adj{r}", bufs=1) as adj_pool, \
         tc.tile_pool(name=f"gt{r}", bufs=1) as gt_pool:
        bias_pp = const_pool.tile([P, CC], F32)
        nc.sync.dma_start(bias_pp[:],
                          biasc_d.ap().rearrange("(cc p) -> p cc", p=P))
        dinvr_bc = const_pool.tile([P, ROWS], F32)

        # adjacency, resident: adj_sb[p, kt, i] = A.T[kt*128+p, i].
        # The DRAM side is host-packed per DMA chunk ([NDQ, P, kq*ROWS]) so
        # every partition's read is one contiguous 4KB line, not a 1KB
        # gather strided by 128KB.
        adj_sb = adj_pool.tile([P, KT, ROWS], FP8)
        kq = KT // NDQ
        for q in range(NDQ):
            nc.sync.dma_start(
                adj_sb[:, q * kq:(q + 1) * kq, :].rearrange(
                    "p k i -> p (k i)"),
                adjt8_d.ap()[q])
        # g, slice-major: slot s*8+c' = global j-tile c'*8+s
        g_t = gt_pool.tile([P, KT, C], BF16)

        if parts == "mm":
            # timing isolation: fill g_t with xt bytes (benign), unit scales
            for b in range(8):
                nc.sync.dma_start(
                    g_t[:, 8 * b:8 * (b + 1), :].rearrange(
                        "p k c -> p (k c)").rearrange("p (f i) -> p f i", f=FT),
                    xt_d.ap().rearrange("(f p) i -> p f i", p=P))
            nc.vector.memset(dinvr_bc[:], 1.0)
        else:
            dinv_all = _emit_pre(nc, tc, dram, io, r, g_t, dinvr_bc,
                                 misc_pool, adj_sb, nch, sched)
            if parts == "pre":
                tok = misc_pool.tile([P, C], F32, tag="tok")
                nc.vector.tensor_copy(tok[:], g_t[:, KT - 1, :])
                nc.sync.dma_start(out_d.ap()[0:P, 0:C], tok[:])
                return

        # ---------------- main matmul + epilogue -----------------------
        stiles = ROWS // nch // P
        cht = NCORES * stiles                  # g_t slots per AllGather chunk
        t8lo = KT - nf8                        # first fp8 slot (nf8 k-tiles)
        assert nf8 % 2 == 0 and (parts == "mm" or stiles % 2 == 0 or
                                 nf8 == 0), "fp8 slots must pair up"

        def slot_kt(t):
            # g_t slot t -> global j-tile (= adjacency k-tile index)
            if parts == "mm":
                return t
            s, rem = divmod(t, cht)
            cp, j = divmod(rem, stiles)
            return cp * MT + s * stiles + j

        if sched == "early" and parts != "mm":
            # post-gather k-scaling: g = dinv_j * h, per slot, DVE runs
            # ahead of the PE's consumption
            for t in range(KT):
                nc.vector.tensor_scalar_mul(
                    g_t[:, t, :], g_t[:, t, :],
                    dinv_all[:, slot_kt(t):slot_kt(t) + 1])

        g8_t = None
        if nf8:
            g8_t = gt_pool.tile([P, (KT - t8lo) // 2, 2, C], FP8)
            for u in range((KT - t8lo) // 2):
                nc.vector.tensor_copy(g8_t[:, u, :, :],
                                      g_t[:, t8lo + 2 * u:t8lo + 2 * u + 2, :])

        with tc.tile_pool(name=f"mmps{r}", bufs=1, space="PSUM") as mmps_pool, \
             tc.tile_pool(name=f"ep{r}", bufs=4) as ep_pool:
            mm_ps = [mmps_pool.tile([P, ROWS], F32, tag=f"mm{cc}",
                                    name=f"mm{cc}")
                     for cc in range(CC)]

            def mm(cc, t, start, stop):
                kt = slot_kt(t)
                for half in range(2):
                    nc.tensor.matmul(
                        mm_ps[cc][:, half * C:(half + 1) * C],
                        lhsT=g_t[:, t, cc * P:(cc + 1) * P],
                        rhs=adj_sb[:, kt, half * C:(half + 1) * C],
                        start=start, stop=stop)

            def mm8(cc, t, start, stop):
                # fp8 DoubleRow over slot pair (t, t+1); adjacency k-tiles
                # (kt, kt+1) are consecutive because stiles is even
                kt = slot_kt(t)
                assert slot_kt(t + 1) == kt + 1
                u = (t - t8lo) // 2
                for half in range(2):
                    nc.tensor.matmul(
                        mm_ps[cc][:, half * C:(half + 1) * C],
                        lhsT=g8_t[:, u, :, cc * P:(cc + 1) * P],
                        rhs=adj_sb[:, kt:kt + 2, half * C:(half + 1) * C],
                        perf_mode=DR,
                        start=start, stop=stop)

            tail0 = min(t8lo, KT - MT)
            for t in range(tail0):
                for cc in range(CC):
                    mm(cc, t, start=(t == 0), stop=False)

            # tail: cc-major so epilogues overlap remaining matmuls
            for cc in range(CC):
                t = tail0
                while t < KT:
                    if nf8 and t >= t8lo:
                        mm8(cc, t, start=False, stop=(t + 2 == KT))
                        t += 2
                    else:
                        mm(cc, t, start=False, stop=(t + 1 == KT))
                        t += 1
                for eh in range(2):
                    sl = slice(eh * C, (eh + 1) * C)
                    t1 = ep_pool.tile([P, C], F32, tag="t1")
                    nc.vector.tensor_mul(t1[:], mm_ps[cc][:, sl],
                                         dinvr_bc[:, sl])
                    t2 = ep_pool.tile([P, C], F32, tag="t2")
                    nc.scalar.activation(
                        t2[:], t1[:], mybir.ActivationFunctionType.Lrelu,
                        bias=bias_pp[:, cc:cc + 1], alpha=0.01)
                    nc.sync.dma_start(
                        out_d.ap()[cc * P:(cc + 1) * P, sl], t2[:])


def _emit_pre(nc, tc, dram, io, r, g_t, dinvr_bc, misc_pool, adj_sb, nch,
              sched):
    """x@W, degree pass, dinv, chunked AllGather -> g_t.

    sched="late": scale own h by own dinv, then gather g (fewest moving
    parts; the first gather chunk waits for the full degree pass).
    sched="early": gather UNSCALED h starting right after x@W, slot the
    tiny dinv AllGather after the first h chunk, and return the all-nodes
    dinv [P, KT] so the caller scales g_t per slot post-gather."""
    adjt8_d, xt_d, w_d, biasc_d, out_d = io
    srows = ROWS // nch                # own rows per AllGather chunk
    stiles = srows // P                # own row tiles per chunk

    g_bounce = dram.tile([ROWS, C], BF16, name=f"g_bounce{r}")
    g_alls = [dram.tile([NCORES * srows, C], BF16, addr_space="Shared",
                        name=f"g_all{r}_{s}") for s in range(nch)]
    dinv_b = dram.tile([1, ROWS], F32, name=f"dinv_b{r}")

    ones8_t = misc_pool.tile([P, 2, 16], FP8, tag="ones8")
    nc.gpsimd.memset(ones8_t[:], 1.0)
    ones1_t = misc_pool.tile([1, P], F32, tag="ones1")
    nc.gpsimd.memset(ones1_t[:], 1.0)
    h_sb = misc_pool.tile([P, MT, C], BF16, tag="hsb")

    with tc.tile_pool(name=f"xw{r}", bufs=1) as xw_pool, \
         tc.tile_pool(name=f"hps{r}", bufs=2, space="PSUM") as hps_pool, \
         tc.tile_pool(name=f"degps{r}", bufs=1, space="PSUM") as degps_pool, \
         tc.tile_pool(name=f"bcps{r}", bufs=1, space="PSUM") as bcps_pool:
        # HAM warm-up: a few dependency-free matmuls fill the PE during
        # the initial DMA wait, so x@W starts at 2.4GHz instead of 1.2
        warm_ps = bcps_pool.tile([P, P], F32, tag="warm")
        for i in range(12):
            nc.tensor.matmul(warm_ps[:], lhsT=ones1_t[:], rhs=ones1_t[:],
                             start=True, stop=True)

        # ---------------- x@W ------------------------------------------
        xt_t = xw_pool.tile([P, FT, ROWS], BF16)
        nc.sync.dma_start(
            xt_t[:], xt_d.ap().rearrange("(f p) i -> p f i", p=P))
        w_t = xw_pool.tile([P, FT, C], BF16)
        nc.sync.dma_start(
            w_t[:], w_d.ap().rearrange("(f p) c -> p f c", p=P))

        for mt in range(MT):
            h_ps = hps_pool.tile([P, C], F32)
            for ft in range(FT):
                nc.tensor.matmul(
                    h_ps[:],
                    lhsT=xt_t[:, ft, mt * P:(mt + 1) * P],
                    rhs=w_t[:, ft, :],
                    start=(ft == 0), stop=(ft == FT - 1))
            nc.scalar.copy(h_sb[:, mt, :], h_ps[:])

        def em_ag(s):
            nc.gpsimd.collective_compute(
                "AllGather", mybir.AluOpType.bypass,
                replica_groups=[list(range(NCORES))],
                ins=[g_bounce[s * srows:(s + 1) * srows, :].opt()],
                outs=[g_alls[s].opt()])
            nc.sync.dma_start(
                g_t[:, s * NCORES * stiles:(s + 1) * NCORES * stiles, :],
                g_alls[s].rearrange("(k p) c -> p k c", p=P))

        if sched == "early":
            # bounce chunk 0 of unscaled h; its gather goes out before the
            # degree pass finishes. Later chunks are dependency-chained
            # behind the dinv AllGather (see below) so the tiny dinv
            # collective gets the 2nd FIFO slot, not the last.
            nc.sync.dma_start(
                g_bounce[0:srows, :].rearrange("(m p) c -> p m c", p=P),
                h_sb[:, 0:stiles, :])
            em_ag(0)

        # ---------------- deg: fp8 DoubleRow over resident adjacency ---
        deg_ps = [degps_pool.tile([1, C], F32, tag=f"degp{i}", name=f"degp{i}")
                  for i in range(2)]
        for q in range(KT // 2):
            for half in range(2):
                nc.tensor.matmul(
                    deg_ps[half][:],
                    lhsT=ones8_t[:, :, 0:1],
                    rhs=adj_sb[:, 2 * q:2 * q + 2, half * C:(half + 1) * C],
                    perf_mode=DR,
                    start=(q == 0), stop=(q == KT // 2 - 1))

        deg_sb = misc_pool.tile([1, ROWS], F32, tag="degsb")
        for half in range(2):
            nc.vector.tensor_copy(
                deg_sb[:, half * C:(half + 1) * C], deg_ps[half][:])
        rrow = misc_pool.tile([1, ROWS], F32, tag="rrow")
        nc.vector.reciprocal(rrow[:], deg_sb[:])
        drow = misc_pool.tile([1, ROWS], F32, tag="drow")
        nc.scalar.sqrt(drow[:], rrow[:])

        # free-axis broadcast of own dinv (epilogue i-scaling)
        bc_ps = bcps_pool.tile([P, ROWS], F32)
        for half in range(2):
            nc.tensor.matmul(
                bc_ps[:, half * C:(half + 1) * C],
                lhsT=ones1_t[:],
                rhs=drow[:, half * C:(half + 1) * C],
                start=True, stop=True)
        nc.vector.tensor_copy(dinvr_bc[:], bc_ps[:])

        if sched == "early":
            # tiny dinv AllGather slots in after the first h chunk, then
            # the remaining h chunks; all-nodes dinv -> [P, KT] via a
            # natural [KT, P] load + PE transpose.
            # dinv_b row 1 is junk copied from the first h chunk's OUTPUT:
            # a real data dependency that forces the scheduler to place
            # this collective after that chunk on the FIFO queue (priority
            # hints alone are ignored).
            dinv_b2 = dram.tile([2, ROWS], F32, name=f"dinv_b2_{r}")
            nc.sync.dma_start(dinv_b2[0:1, :], drow[:])
            nc.gpsimd.dma_start(dinv_b2[1:2, 0:C // 2],
                                g_alls[0][0:1, 0:C // 2])
            dinv_all_b = dram.tile([NCORES * 2, ROWS], F32,
                                   addr_space="Shared", name=f"dinv_all{r}")
            nc.gpsimd.collective_compute(
                "AllGather", mybir.AluOpType.bypass,
                replica_groups=[list(range(NCORES))],
                ins=[dinv_b2.opt()], outs=[dinv_all_b.opt()])
            for s in range(1, nch):
                # junk write from the dinv gather's output into this
                # chunk's bounce region (immediately overwritten by the
                # real bounce): forces chunk s behind the dinv collective
                nc.gpsimd.dma_start(
                    g_bounce[s * srows:s * srows + 1, 0:2],
                    dinv_all_b[0:1, 0:2])
                nc.sync.dma_start(
                    g_bounce[s * srows:(s + 1) * srows, :].rearrange(
                        "(m p) c -> p m c", p=P),
                    h_sb[:, s * stiles:(s + 1) * stiles, :])
                em_ag(s)

            dinvkt_sb = misc_pool.tile([KT, P], F32, tag="dinvkt")
            for cp in range(NCORES):
                nc.sync.dma_start(
                    dinvkt_sb[cp * MT:(cp + 1) * MT, :],
                    dinv_all_b[2 * cp:2 * cp + 1, :].rearrange(
                        "one (m p) -> (one m) p", p=P))
            ident_t = misc_pool.tile([KT, KT], F32, tag="ident")
            make_identity(nc, ident_t[:])
            tp_ps = bcps_pool.tile([P, KT], F32, tag="tp")
            nc.tensor.transpose(tp_ps[:], dinvkt_sb[:], ident_t[:])
            dinv_all = misc_pool.tile([P, KT], F32, tag="dinvall")
            nc.vector.tensor_copy(dinv_all[:], tp_ps[:])
            return dinv_all

        # sched == "late":
        # per-partition dinv for the k-scaling of own h rows:
        # dinv_pm[p, mt] = drow[mt*128 + p] via a DRAM round-trip
        nc.sync.dma_start(dinv_b[:], drow[:])
        dinv_pm = misc_pool.tile([P, MT], F32, tag="dinvpm")
        nc.sync.dma_start(
            dinv_pm[:],
            dinv_b.rearrange("one (m p) -> p (one m)", p=P))

        # ---------------- scale + bounce + AllGather, per chunk --------
        for s in range(nch):
            for j in range(stiles):
                mt = s * stiles + j
                nc.vector.tensor_scalar_mul(
                    h_sb[:, mt, :], h_sb[:, mt, :], dinv_pm[:, mt:mt + 1])
            nc.sync.dma_start(
                g_bounce[s * srows:(s + 1) * srows, :].rearrange(
                    "(m p) c -> p m c", p=P),
                h_sb[:, s * stiles:(s + 1) * stiles, :])
            em_ag(s)
        return None


def build_kernel(reps: int = 1, parts: str = "all", nch: int = 4,
                 nf8: int = 24, sched: str = "early"):
    """Build and compile the SPMD Bass program (identical on all 8 cores)."""
    nc = bacc.Bacc("TRN2", target_bir_lowering=False, debug=False,
                   num_devices=NCORES)

    adjt8_d = nc.dram_tensor("adjt8", [NDQ, P, (KT // NDQ) * ROWS], FP8,
                             kind="ExternalInput")
    xt_d = nc.dram_tensor("xt", [C, ROWS], BF16, kind="ExternalInput")
    w_d = nc.dram_tensor("w", [C, C], BF16, kind="ExternalInput")
    biasc_d = nc.dram_tensor("biasc", [C], F32, kind="ExternalInput")
    out_d = nc.dram_tensor("out", [C, ROWS], F32, kind="ExternalOutput")
    io = (adjt8_d, xt_d, w_d, biasc_d, out_d)

    with tile.TileContext(nc) as tc:
        with tc.tile_pool(name="dram", bufs=1, space="DRAM") as dram:
            if reps == 0:
                with tc.tile_pool(name="nullp", bufs=1) as np_pool:
                    z = np_pool.tile([P, CC], F32)
                    nc.sync.dma_start(
                        z[:], biasc_d.ap().rearrange("(cc p) -> p cc", p=P))
            for r in range(reps):
                _emit(nc, tc, dram, io, r, parts=parts, nch=nch, nf8=nf8,
                      sched=sched)

    nc.compile()
    return nc


def prepare_inputs(x, adj, weightr, weightd, bias):
    """Host-side sharding/layout. Returns in_maps for the 8 cores."""
    x = np.asarray(x, dtype=np.float32)
    adj = np.asarray(adj, dtype=np.float32)
    weightr = np.asarray(weightr, dtype=np.float32)
    weightd = np.asarray(weightd, dtype=np.float32)
    bias = np.ascontiguousarray(np.asarray(bias, dtype=np.float32))

    wr16 = weightr.astype(ml_dtypes.bfloat16)
    wd16 = weightd.astype(ml_dtypes.bfloat16)
    idx = np.arange(ROWS)
    # A values are only 0/1/2: build uint8 once, then LUT-cast (fast + exact)
    lut8 = np.array([0x00, 0x38, 0x40], dtype=np.uint8)  # e4m3 bits for 0/1/2

    in_maps = []
    for c in range(NCORES):
        rows = slice(c * ROWS, (c + 1) * ROWS)
        ai = adj[rows, :].T.astype(np.uint8)             # [N, ROWS] 0/1
        ai[c * ROWS + idx, idx] += 1                     # fold in self-loop
        kq = KT // NDQ
        adjt8 = (lut8[ai].view(ml_dtypes.float8_e4m3)
                 .reshape(NDQ, kq, P, ROWS).transpose(0, 2, 1, 3)
                 .reshape(NDQ, P, kq * ROWS))
        adjt8 = np.ascontiguousarray(adjt8)
        xt = np.ascontiguousarray(x[rows, :].T).astype(ml_dtypes.bfloat16)
        w = wr16 if c < NCORES // 2 else wd16
        in_maps.append({"adjt8": adjt8, "xt": xt, "w": w, "biasc": bias})
    return in_maps


_NC_CACHE = {}


def kernel(x, adj, weightr, weightd, bias):
    if "nc" not in _NC_CACHE:
        _NC_CACHE["nc"] = build_kernel(reps=1)
    nc = _NC_CACHE["nc"]
    in_maps = prepare_inputs(x, adj, weightr, weightd, bias)
    res = run_bass_kernel_spmd(nc, in_maps, list(range(NCORES)))
    out = np.concatenate(
        [np.ascontiguousarray(res.results[c]["out"].T) for c in range(NCORES)],
        axis=0)
    return out
